# revision 15
# baseline (speedup 1.0000x reference)
"""Trainium2 Bass kernel for nn_Block_softmoe (dense transformer block, B=4 S=2048 C=256 H=8).

Strategy
--------
Sharding: 8 cores = (batch b, query-half). Each core computes the full block for
1024 query rows of one batch. K/V are computed per-core over that batch's keys
(2x redundant K/V projection; tiny at dim 256). No collectives.

Mask compaction: the key mask (Bernoulli 0/1) is applied on the host by
gathering only the kept key rows (~1024 of 2048). This halves the dominant
cost, the softmax exp on the ScalarE (ACT) engine, plus the score/attnV
matmuls. Padding rows up to L (multiple of 128) are killed with a -1e30 exp
bias so they contribute exactly 0, matching the reference's -inf masking.

Dataflow (all feature-major / "transposed", so no transposes are needed):
  xT [C, S]  (host-transposed)
  QT = WqT.T @ xqT       [256, 1024]  (feature-major)
  KT = WkT.T @ xkT       [256, L]
  V  = xkT.T @ WvT       [L, 256]    (token-major; lhsT for attnV)
  per head h: ST[kc] = KT_h[:,kc].T @ QT_h   [128, 1024] in PSUM  (contraction hd=32)
              PT[h][kc] = exp(scale*ST + maskbias)  -> SBUF bf16  (ACT, the bottleneck)
  attnV: 4-head col-tiled matmuls accumulate  O^T[32j:32j+32,:] += V_h[kc].T @ PT[h][kc]
  sums:  M=1 col-tiled ones-matmuls           S[32j,:] += 1.T @ PT[h][kc]
  softmax normalize: recip = 1/S (DVE), broadcast via E128 matmul (PE is the
  only partition-broadcast engine), xoutT = O^T * recip_bcast  (DVE)
  MLP: h1T = gelu(W1T.T @ xoutT + b1) (exact Gelu on ACT)
  final[s,:] = [h1T; xoutT].T @ [W2T; I]  -- residual fused via identity block
Biases: bq/bk folded into the projection PSUM->SBUF copies (per-partition
tensor_scalar add), b1 into the Gelu activation bias, bv added per-partition
after normalization (only if nonzero), b2 added on host (pure output offset).

Self-contained: hardcodes all shapes; compiled NEFF cached per L.
"""

import os
import sys

for _p in ("/opt/trn_rl_repo", "/root/.axon_site/_ro/trn_rl_repo"):
    if os.path.isdir(_p) and _p not in sys.path:
        sys.path.append(_p)

import numpy as np

import concourse.bacc as bacc
import concourse.tile as tile
from concourse import mybir
from concourse.bass_utils import run_bass_kernel_spmd

B, S, C, H, HD = 4, 2048, 256, 8, 32
NCORES = 8
SQ = 1024                      # query rows per core
SCALE = float(HD) ** -0.5
F32 = mybir.dt.float32
BF16 = mybir.dt.bfloat16
AF = mybir.ActivationFunctionType
NEG = -1e30

_cache: dict = {}

F32R = mybir.dt.float32r



def _build(L: int, use_bv: bool, stop_after: str | None = None):
    """Build the single-core program (SPMD across 8 cores)."""
    KC = L // 128
    nc = bacc.Bacc("TRN2", target_bir_lowering=False, debug=False, num_devices=NCORES)

    # ---- I/O ----
    d_xqT = nc.dram_tensor("xqT", [C, SQ], F32R, kind="ExternalInput")
    d_xkT = nc.dram_tensor("xkT", [C, L], F32R, kind="ExternalInput")
    d_wqT = nc.dram_tensor("wqT", [C, C], F32R, kind="ExternalInput")
    d_wkT = nc.dram_tensor("wkT", [C, C], F32R, kind="ExternalInput")
    d_wvT = nc.dram_tensor("wvT", [C, C], F32R, kind="ExternalInput")
    d_w1T = nc.dram_tensor("w1T", [C, C], F32R, kind="ExternalInput")
    d_w2TA = nc.dram_tensor("w2TA", [2 * C, C], F32R, kind="ExternalInput")
    d_mb = nc.dram_tensor("mb", [128, KC], F32, kind="ExternalInput")
    d_e128 = nc.dram_tensor("e128", [128, 128], F32, kind="ExternalInput")
    d_bqk1 = nc.dram_tensor("bqk1", [128, 6], F32, kind="ExternalInput")  # bq|bk|b1 chunks
    d_bv = nc.dram_tensor("bv", [128, 2], F32, kind="ExternalInput")
    d_out = nc.dram_tensor("out", [SQ, C], F32, kind="ExternalOutput")

    kchunks = [(o, min(512, L - o)) for o in range(0, L, 512)]

    with tile.TileContext(nc) as tc:
        with tc.tile_pool(name="persist", bufs=1) as pp, \
             tc.tile_pool(name="pt", bufs=1) as ptp, \
             tc.tile_pool(name="work", bufs=2) as wp, \
             tc.tile_pool(name="ps_s", bufs=2, space="PSUM") as ps_s, \
             tc.tile_pool(name="ps_a", bufs=1, space="PSUM") as ps_a, \
             tc.tile_pool(name="ps_m", bufs=1, space="PSUM") as ps_m:

            # ---- load inputs ----
            xqT = [pp.tile([128, SQ], F32R, tag=f"xqT{m}", name=f"xqT{m}") for m in range(2)]
            xkT = [pp.tile([128, L], F32R, tag=f"xkT{m}", name=f"xkT{m}") for m in range(2)]
            wqT = [pp.tile([128, C], F32R, tag=f"wqT{m}", name=f"wqT{m}") for m in range(2)]
            wkT = [pp.tile([128, C], F32R, tag=f"wkT{m}", name=f"wkT{m}") for m in range(2)]
            wvT = [pp.tile([128, C], F32R, tag=f"wvT{m}", name=f"wvT{m}") for m in range(2)]
            w1T = [pp.tile([128, C], F32R, tag=f"w1T{m}", name=f"w1T{m}") for m in range(2)]
            w2TA = [pp.tile([128, C], F32R, tag=f"w2TA{m}", name=f"w2TA{m}") for m in range(4)]
            # score-path weights first so the K/Q projections start ASAP
            for m in range(2):
                nc.sync.dma_start(out=wqT[m], in_=d_wqT[m * 128:(m + 1) * 128, :])
                nc.sync.dma_start(out=wkT[m], in_=d_wkT[m * 128:(m + 1) * 128, :])
            for m in range(2):
                nc.sync.dma_start(out=xkT[m], in_=d_xkT[m * 128:(m + 1) * 128, :])
                nc.sync.dma_start(out=xqT[m], in_=d_xqT[m * 128:(m + 1) * 128, :])
            for m in range(2):
                nc.sync.dma_start(out=wvT[m], in_=d_wvT[m * 128:(m + 1) * 128, :])
                nc.sync.dma_start(out=w1T[m], in_=d_w1T[m * 128:(m + 1) * 128, :])
            for m in range(4):
                nc.sync.dma_start(out=w2TA[m], in_=d_w2TA[m * 128:(m + 1) * 128, :])
            mb = pp.tile([128, KC], F32, tag="mb")
            nc.sync.dma_start(out=mb, in_=d_mb[:, :])
            e128 = pp.tile([128, 128], F32, tag="e128")
            nc.sync.dma_start(out=e128, in_=d_e128[:, :])
            bqk1 = pp.tile([128, 6], F32, tag="bqk1")
            nc.sync.dma_start(out=bqk1, in_=d_bqk1[:, :])
            bv = pp.tile([128, 2], F32, tag="bv")
            nc.sync.dma_start(out=bv, in_=d_bv[:, :])
            ones = pp.tile([128, 1], BF16, tag="ones")
            nc.vector.memset(ones, 1.0)

            # ---- projections (feature-major QT/KT, token-major V) ----
            QT = [pp.tile([128, SQ], F32R, tag=f"QT{m}", name=f"QT{m}") for m in range(2)]
            KT = [pp.tile([128, L], F32R, tag=f"KT{m}", name=f"KT{m}") for m in range(2)]
            V = [pp.tile([128, C], BF16, tag=f"V{sc}", name=f"V{sc}") for sc in range(KC)]

            alt = [0]
            def _ps():
                alt[0] ^= 1
                return ps_m.tile([128, 512], F32, tag=("proj" if alt[0] else "prb"),
                                 name="pp")
            for m in range(2):
                for n in range(2):  # SQ/512
                    pq = _ps()
                    for kk in range(2):
                        nc.tensor.matmul(out=pq, lhsT=wqT[kk][:, m * 128:(m + 1) * 128],
                                         rhs=xqT[kk][:, n * 512:(n + 1) * 512],
                                         start=(kk == 0), stop=(kk == 1))
                    nc.vector.tensor_scalar_add(out=QT[m][:, n * 512:(n + 1) * 512],
                                                in0=pq, scalar1=bqk1[:, m:m + 1])
                for o, w in kchunks:
                    pk = _ps()
                    for kk in range(2):
                        nc.tensor.matmul(out=pk[:, :w], lhsT=wkT[kk][:, m * 128:(m + 1) * 128],
                                         rhs=xkT[kk][:, o:o + w],
                                         start=(kk == 0), stop=(kk == 1))
                    nc.vector.tensor_scalar_add(out=KT[m][:, o:o + w], in0=pk[:, :w],
                                                scalar1=bqk1[:, 2 + m:3 + m])
            for sc in range(KC):
                pv = _ps()
                for kk in range(2):
                    nc.tensor.matmul(out=pv[:, :C], lhsT=xkT[kk][:, sc * 128:(sc + 1) * 128],
                                     rhs=wvT[kk][:, :], start=(kk == 0), stop=(kk == 1))
                nc.vector.tensor_copy(out=V[sc], in_=pv[:, :C])

            if stop_after == "proj":
                return_early = True
            # ---- attention ----
            # Program order interleaves group 0's attnV/sums phase with group
            # 1's scores so the ACT engine (the bottleneck, running exps)
            # never starves while the PE drains a group's attnV accumulation.
            xoutT = [pp.tile([128, SQ], F32R, tag=f"xoutT{g}", name=f"xoutT{g}") for g in range(2)]
            PT = {}

            def emit_scores(g, kc):
                # adjacent heads hit different PE row-groups -> concurrent strips
                for j in range(4):
                    h = 4 * g + j
                    pss = ps_s.tile([128, SQ], F32, tag="scores", name="pss")
                    for qn in range(2):
                        nc.tensor.matmul(
                            out=pss[:, qn * 512:(qn + 1) * 512],
                            lhsT=KT[g][32 * j:32 * j + 32, kc * 128:(kc + 1) * 128],
                            rhs=QT[g][32 * j:32 * j + 32, qn * 512:(qn + 1) * 512],
                            start=True, stop=True,
                            tile_position=(32 * j, 0))
                    pt_t = ptp.tile([128, SQ], BF16, tag="pt", bufs=45,
                                    name=f"pt{h}_{kc}")
                    nc.scalar.activation(out=pt_t, in_=pss, func=AF.Exp,
                                         bias=mb[:, kc:kc + 1], scale=SCALE)
                    PT[h, kc] = pt_t

            def open_attn():
                po = ps_a.tile([128, 512], F32, tag="po", name="po")
                psum = ps_a.tile([128, 512], F32, tag="psum", name="psum")
                nc.vector.memset(psum, 1.0)
                return po, psum

            def emit_attn(g, qc, kc, po, psum):
                # 4 col-strips back-to-back -> concurrent on the PE array
                for j in range(4):
                    h = 4 * g + j
                    nc.tensor.matmul(out=po[32 * j:32 * j + 32, :],
                                     lhsT=V[kc][:, h * 32:(h + 1) * 32],
                                     rhs=PT[h, kc][:, qc * 512:(qc + 1) * 512],
                                     start=(kc == 0), stop=(kc == KC - 1),
                                     tile_position=(0, 32 * j),
                                     skip_group_check=(j > 0))
                for j in range(4):
                    h = 4 * g + j
                    nc.tensor.matmul(out=psum[32 * j:32 * j + 1, :],
                                     lhsT=ones[:, 0:1],
                                     rhs=PT[h, kc][:, qc * 512:(qc + 1) * 512],
                                     start=(kc == 0), stop=(kc == KC - 1),
                                     tile_position=(0, 32 * j),
                                     skip_group_check=(j > 0))

            def emit_normalize(g, qc, po, psum):
                rec = wp.tile([128, 512], F32, tag="rec", name="rec")
                nc.vector.reciprocal_approx_fast(out=rec, in_=psum)
                prb = _ps()
                nc.tensor.matmul(out=prb, lhsT=e128, rhs=rec, start=True, stop=True)
                rb = wp.tile([128, 512], F32, tag="rb", name="rb")
                nc.vector.tensor_copy(out=rb, in_=prb)
                xo = xoutT[g][:, qc * 512:(qc + 1) * 512]
                nc.vector.tensor_mul(out=xo, in0=po, in1=rb)
                if use_bv:
                    nc.vector.tensor_scalar_add(out=xo, in0=xo,
                                                scalar1=bv[:, g:g + 1])

            if stop_after != "proj":
                for kc in range(KC):
                    emit_scores(0, kc)
                if stop_after != "scores":
                    # group 0 attnV interleaved with group 1 scores
                    units = [("a", 0, qc, kc) for qc in range(2) for kc in range(KC)]
                    sidx = 0
                    cur = None
                    for i, (_, g, qc, kc) in enumerate(units):
                        if i % 2 == 0 and sidx < KC:
                            emit_scores(1, sidx)
                            sidx += 1
                        if kc == 0:
                            cur = open_attn()
                        emit_attn(g, qc, kc, *cur)
                        if kc == KC - 1:
                            emit_normalize(g, qc, *cur)
                    while sidx < KC:
                        emit_scores(1, sidx)
                        sidx += 1
                    # group 1 attnV (tail)
                    for qc in range(2):
                        cur = open_attn()
                        for kc in range(KC):
                            emit_attn(1, qc, kc, *cur)
                        emit_normalize(1, qc, *cur)

            # ---- MLP + fused residual ----
            if stop_after in ("proj", "scores", "attn"):
                h1_range = ()
                f_range = ()
            else:
                h1_range = range(2)
                f_range = range(8)
            h1T = [pp.tile([128, SQ], F32R, tag=f"h1T{j}", name=f"h1T{j}") for j in range(2)]
            for n in ([] if not h1_range else range(2)):
                for j in range(2):
                    ph = _ps()
                    for cc in range(2):
                        nc.tensor.matmul(out=ph, lhsT=w1T[cc][:, j * 128:(j + 1) * 128],
                                         rhs=xoutT[cc][:, n * 512:(n + 1) * 512],
                                         start=(cc == 0), stop=(cc == 1))
                    nc.scalar.activation(out=h1T[j][:, n * 512:(n + 1) * 512], in_=ph,
                                         func=AF.Gelu, bias=bqk1[:, 4 + j:5 + j])
                for sc in range(4 * n, 4 * n + 4):
                    pf = _ps()
                    for cc in range(4):
                        lh = h1T[cc] if cc < 2 else xoutT[cc - 2]
                        nc.tensor.matmul(out=pf[:, :C],
                                         lhsT=lh[:, sc * 128:(sc + 1) * 128],
                                         rhs=w2TA[cc][:, :],
                                         start=(cc == 0), stop=(cc == 3))
                    ot = wp.tile([128, C], F32, tag="ot", name="ot")
                    nc.vector.tensor_copy(out=ot, in_=pf[:, :C])
                    nc.sync.dma_start(out=d_out[sc * 128:(sc + 1) * 128, :], in_=ot)

    nc.compile()
    return nc


def _prep_inputs(x, mask, Wq, bq, Wk, bk, Wv, bv, W1, b1, W2, b2):
    """Host-side sharding + layout prep. Returns (L, in_maps, use_bv)."""
    x = np.ascontiguousarray(x, dtype=np.float32)
    keeps = [np.flatnonzero(mask[b, :S] != 0) for b in range(B)]
    cnts = [len(k) for k in keeps]
    L = max(128, -(-max(cnts) // 128) * 128)
    KC = L // 128

    wqT = np.ascontiguousarray(Wq.T, dtype=np.float32)
    wkT = np.ascontiguousarray(Wk.T, dtype=np.float32)
    wvT = np.ascontiguousarray(Wv.T, dtype=np.float32)
    w1T = np.ascontiguousarray(W1.T, dtype=np.float32)
    w2TA = np.ascontiguousarray(
        np.vstack([W2.T.astype(np.float32), np.eye(C, dtype=np.float32)]))
    e128 = np.zeros((128, 128), dtype=np.float32)
    for m in range(128):
        e128[32 * (m // 32), m] = 1.0
    bqk1 = np.stack([
        bq[0:128], bq[128:256], bk[0:128], bk[128:256], b1[0:128], b1[128:256],
    ], axis=1).astype(np.float32)
    bv_t = np.stack([bv[0:128], bv[128:256]], axis=1).astype(np.float32)
    use_bv = bool(np.any(bv != 0))

    in_maps = []
    for core in range(NCORES):
        b, half = core // 2, core % 2
        xb = x[b]                                   # [S, C]
        xqT = np.ascontiguousarray(xb[half * SQ:(half + 1) * SQ].T)   # [C, SQ]
        xk = np.zeros((L, C), dtype=np.float32)
        xk[:cnts[b]] = xb[keeps[b]]
        xkT = np.ascontiguousarray(xk.T)            # [C, L]
        mb = np.full(L, NEG, dtype=np.float32)
        mb[:cnts[b]] = 0.0
        mb = np.ascontiguousarray(mb.reshape(KC, 128).T)  # [128, KC]
        in_maps.append({
            "xqT": xqT, "xkT": xkT, "wqT": wqT, "wkT": wkT, "wvT": wvT,
            "w1T": w1T, "w2TA": w2TA, "mb": mb, "e128": e128,
            "bqk1": bqk1, "bv": bv_t,
        })
    return L, in_maps, use_bv


def kernel(x, mask, Wq, bq, Wk, bk, Wv, bv, W1, b1, W2, b2):
    L, in_maps, use_bv = _prep_inputs(x, mask, Wq, bq, Wk, bk, Wv, bv, W1, b1, W2, b2)
    key = (L, use_bv)
    if key not in _cache:
        _cache[key] = _build(L, use_bv)
    nc = _cache[key]
    res = None
    last_exc = None
    for attempt in range(4):
        try:
            res = run_bass_kernel_spmd(nc, in_maps, core_ids=list(range(NCORES)),
                                       trace=False)
            break
        except Exception as e:  # transient device errors on first exec of a NEFF
            last_exc = e
            import time as _time
            import jax as _jax
            _time.sleep(2.0)
            try:
                _jax.clear_caches()
            except Exception:
                pass
    if res is None:
        raise last_exc
    out = np.empty((B, S, C), dtype=np.float32)
    for core in range(NCORES):
        b, half = core // 2, core % 2
        out[b, half * SQ:(half + 1) * SQ] = res.results[core]["out"]
    if np.any(b2 != 0):
        out += np.asarray(b2, dtype=np.float32)[None, None, :]
    # stash for test harness reuse (timing reruns)
    kernel.last = {"nc": nc, "in_maps": in_maps, "L": L}
    return out
